# revision 47
# baseline (speedup 1.0000x reference)
"""Expert-parallel MoE kernel for Trainium2 (8 NeuronCores), v2.

Reference computation (dense in the reference, but top-2 sparse in effect):
  scores = softmax(x @ gate_w + gate_b)          [B,T,E]
  keep top-2 per token, L1-renormalize -> g      [B,T,E] (only 2 nonzero)
  out = sum_e g[:,e] * (relu(x@w1[e]+b1[e]) @ w2[e] + b2[e])

Strategy (all compute on device):
  - Core e owns expert e (weights sharded along E).
  - Gating is token-sharded: core i computes full top-2 gates for tokens
    [i*NTOK/8, (i+1)*NTOK/8) in fp32, packs each token's two picks as
    (expert_idx + gate_fraction) into a [NTOK, 2] f32 AllGather payload.
  - Tokens are processed in 4 "quarters" (token ranges of NTOK/4). Each
    core compacts the token ids routed to its expert per quarter
    (sparse_gather of iota+gate packed values), gathers those token rows
    of x (bf16) with a transposing dma_gather, runs the two matmuls in
    bf16 (fp32 accumulation), scales by the gate values and scatter-adds
    into a per-quarter zeroed [QT, O] partial (bf16, zeroed by DMAs that
    are dependency-gated to run during the MLP window).
  - Four ReduceScatters (one per quarter, issued as soon as that
    quarter's scatter-adds land) sum the partials across cores and write
    directly into interleaved blocks of the y output; the host
    reassembles the block permutation. Only the last RS sits on the
    critical path.
"""

import numpy as np
import ml_dtypes

import concourse.bacc as bacc
import concourse.bass as bass
import concourse.mybir as mybir
import concourse.tile as tile
from concourse.bass_utils import run_bass_kernel_spmd

F32 = mybir.dt.float32
BF16 = mybir.dt.bfloat16
I16 = mybir.dt.int16
U32 = mybir.dt.uint32
AX = mybir.AxisListType
ALU = mybir.AluOpType
ACT = mybir.ActivationFunctionType

# Full-problem constants (hardcoded per the harness contract).
# SLOTS_Q: computed capacity per token-quarter (max routed count for the
# fixed seed-0 inputs is 559 per (expert, quarter); margin 17).
FULL = dict(B=4, T=2048, D=1024, H=2048, O=1024, E=8, SLOTS_Q=576)
N_CORES = 8
NQ = 4  # token quarters


def _chunks(slots):
    out = []
    off = 0
    while off < slots:
        c = min(512, slots - off)
        out.append((off, c))
        off += c
    return out


DEBUG_DUMP = False


def build(cfg=FULL, with_b2=False, warmup=4):
    B, T, D, H, O, E = cfg["B"], cfg["T"], cfg["D"], cfg["H"], cfg["O"], cfg["E"]
    SLOTS_Q = cfg["SLOTS_Q"]
    NTOK = B * T
    KD = D // 128           # K-tiles in D
    KH = H // 128           # K-tiles in H
    MH = H // 128           # M-tiles for layer 1
    NO = O // 512           # 512-wide N-chunks in O for layer 2
    TSL = NTOK // N_CORES   # gating token slice per core
    JSL = TSL // 128        # token tiles in my gating slice
    JALL = NTOK // 128      # free-dim length of token-major [128, JALL] tiles
    QT = NTOK // NQ         # tokens per quarter
    QP = 128 // NQ          # partitions per quarter in [128, JALL] layout
    SH = TSL // NQ          # per-core y rows per quarter (RS shard)
    NTILE = (SLOTS_Q + 127) // 128   # 128-slot tiles per quarter
    QCAP = 128 * NTILE               # compaction array capacity
    FCAP = QCAP // 16                # sparse_gather output cols
    CH = _chunks(SLOTS_Q)
    assert FCAP % 8 == 0 and QT % 16 == 0 and TSL % NQ == 0

    nc = bacc.Bacc("TRN2", target_bir_lowering=False, debug=False,
                   num_devices=N_CORES)

    # ---- I/O ----
    xT = nc.dram_tensor("xT", [128, KD, TSL], F32, kind="ExternalInput")
    gw = nc.dram_tensor("gw", [128, KD, E], F32, kind="ExternalInput")
    gb = nc.dram_tensor("gb", [E, 1], F32, kind="ExternalInput")
    xbf = nc.dram_tensor("xbf", [NTOK, D], BF16, kind="ExternalInput")
    w1 = nc.dram_tensor("w1", [128, KD, H], BF16, kind="ExternalInput")
    b1 = nc.dram_tensor("b1", [128, MH], F32, kind="ExternalInput")
    w2 = nc.dram_tensor("w2", [128, KH, O], BF16, kind="ExternalInput")
    b2 = nc.dram_tensor("b2", [1, O], BF16, kind="ExternalInput")
    eid = nc.dram_tensor("eid", [128, 1], F32, kind="ExternalInput")
    y = nc.dram_tensor("y", [TSL, O], BF16, kind="ExternalOutput")
    # Per-quarter partial accumulators (internal: NeuronCC forbids
    # collectives reading IO tensors). Rows [0, QT) are zeroed on device
    # off the critical path; +128 trash rows take the capacity-padding
    # scatter-adds and are never read.
    parts = [nc.dram_tensor(f"partial{q}", [QT + 128, O], BF16)
             for q in range(NQ)]

    # ---- constants (embedded in NEFF) ----
    # token-major iota for the partition-major post-AG layout:
    # token t = p * JALL + j
    iota_np = (np.arange(128)[:, None] * JALL
               + np.arange(JALL)[None, :]).astype(np.float32)
    iota_c = nc.inline_tensor(iota_np, name="iota_c")
    iota8_np = np.tile(np.arange(E, dtype=np.float32), (128, 1))
    iota8_c = nc.inline_tensor(iota8_np, name="iota8_c")
    id8_c = nc.inline_tensor(np.eye(E, dtype=np.float32), name="id8_c")
    ones_c = nc.inline_tensor(np.ones((1, 128), dtype=ml_dtypes.bfloat16),
                              name="ones_c")
    sel16_np = np.tile(np.eye(16, dtype=np.float32), (1, 8))
    sel16_c = nc.inline_tensor(sel16_np, name="sel16_c")
    posc_np = (np.arange(FCAP)[None, :] * 16
               + np.arange(16)[:, None]).astype(np.float32)
    posc_c = nc.inline_tensor(posc_np, name="posc_c")

    # ---- internal DRAM (collective operands) ----
    ag_in = nc.dram_tensor("ag_in", [TSL, 2], F32)
    ag_out = nc.dram_tensor("ag_out", [NTOK, 2], F32)
    rs_out = nc.dram_tensor("rs_out", [TSL, O], BF16)
    nfd = [nc.dram_tensor(f"nfd{q}", [1, 1], F32) for q in range(NQ)]

    groups = [list(range(N_CORES))]

    with tile.TileContext(nc) as tc:
        with (
            tc.tile_pool(name="persist", bufs=1) as pp,
            tc.tile_pool(name="stream", bufs=5) as sp,
            tc.tile_pool(name="xgp", bufs=4) as xp,
            tc.tile_pool(name="hbuf", bufs=2) as hp,
            tc.tile_pool(name="outb", bufs=2) as ob,
            tc.tile_pool(name="psA", bufs=2, space="PSUM") as psA,
            tc.tile_pool(name="psG", bufs=2, space="PSUM") as psG,
            tc.tile_pool(name="psB", bufs=1, space="PSUM") as psB,
            tc.tile_pool(name="psC", bufs=2, space="PSUM") as psC,
        ):
            # ---- latency-critical consts for gating (SP queue) ----
            gws = pp.tile([128, KD, E], F32, tag="gws")
            nc.sync.dma_start(gws[:], gw[:])
            gbs = pp.tile([E, 1], F32, tag="gbs")
            nc.sync.dma_start(gbs[:], gb[:])
            id8s = pp.tile([E, E], F32, tag="id8s")
            nc.sync.dma_start(id8s[:], id8_c[:])

            GC = min(512, TSL)

            # ---- PE warmup: garbage matmuls to climb the p-state ramp
            # while the gating activations stream in ----
            if warmup:
                wt = pp.tile([128, GC], BF16, tag="warm")
                nc.vector.memset(wt[:], 0.0)
                for _ in range(warmup):
                    pw = psG.tile([E, GC], F32, tag="ps_gate")
                    nc.tensor.matmul(pw[:], wt[:, 0:E], wt[:],
                                     start=True, stop=True)

            # ---------- gating for my token slice (fp32) ----------
            stok = pp.tile([128, JSL, E], F32, tag="stok")
            for nch2 in range(TSL // GC):
                ps = psG.tile([E, GC], F32, tag="ps_gate")
                for k in range(KD):
                    xk = sp.tile([128, GC], F32, tag="xk")
                    nc.sync.dma_start(
                        xk[:], xT[:, k, nch2 * GC:(nch2 + 1) * GC])
                    nc.tensor.matmul(ps[:], gws[:, k, :], xk[:],
                                     start=(k == 0), stop=(k == KD - 1))
                sct = sp.tile([E, GC], F32, tag="sct")
                nc.vector.tensor_scalar_add(sct[:], ps[:], gbs[:])
                for tt in range(GC // 128):
                    pst = psB.tile([128, E], F32, tag="pst")
                    nc.tensor.matmul(
                        pst[:], sct[:, tt * 128:(tt + 1) * 128], id8s[:],
                        start=True, stop=True)
                    nc.vector.tensor_copy(
                        stok[:, nch2 * (GC // 128) + tt, :], pst[:])

            # consts used after gating (emitted behind the xk loads on SP)
            iota8s = pp.tile([128, E], F32, tag="iota8s")
            nc.sync.dma_start(iota8s[:], iota8_c[:])
            eids = pp.tile([128, 1], F32, tag="eids")
            nc.sync.dma_start(eids[:], eid[:])
            iotas = pp.tile([128, JALL], F32, tag="iotas")
            nc.sync.dma_start(iotas[:], iota_c[:])
            sel16s = pp.tile([16, 128], F32, tag="sel16s")
            nc.sync.dma_start(sel16s[:], sel16_c[:])
            poscs = pp.tile([16, FCAP], F32, tag="poscs")
            nc.sync.dma_start(poscs[:], posc_c[:])

            # -------- top-2 + packed (idx + gate) for my slice --------
            l1 = pp.tile([128, JSL], F32, tag="l1")
            nc.vector.reduce_max(l1[:], stok[:], axis=AX.X)
            l1b = l1[:].unsqueeze(-1).broadcast_to([128, JSL, E])
            eq = pp.tile([128, JSL, E], F32, tag="eq")
            nc.vector.tensor_tensor(eq[:], stok[:], l1b, op=ALU.is_equal)
            msc = pp.tile([128, JSL, E], F32, tag="msc")
            nc.vector.tensor_scalar_mul(msc[:], eq[:], -1e30)
            nc.vector.tensor_add(msc[:], msc[:], stok[:])  # masked scores
            l2 = pp.tile([128, JSL], F32, tag="l2")
            nc.vector.reduce_max(l2[:], msc[:], axis=AX.X)
            l2b = l2[:].unsqueeze(-1).broadcast_to([128, JSL, E])
            # idx1 = argmax, idx2 = arg-2nd-max via iota dot products
            i8b = iota8s[:].unsqueeze(1).broadcast_to([128, JSL, E])
            tmp = pp.tile([128, JSL, E], F32, tag="tmp")
            nc.vector.tensor_tensor(tmp[:], eq[:], i8b, op=ALU.mult)
            idx1 = pp.tile([128, JSL], F32, tag="idx1")
            nc.vector.reduce_sum(idx1[:], tmp[:], axis=AX.X)
            msk2 = pp.tile([128, JSL, E], F32, tag="msk2")
            nc.vector.tensor_tensor(msk2[:], stok[:], l2b, op=ALU.is_ge)
            nc.vector.tensor_sub(msk2[:], msk2[:], eq[:])
            nc.vector.tensor_tensor(tmp[:], msk2[:], i8b, op=ALU.mult)
            idx2 = pp.tile([128, JSL], F32, tag="idx2")
            nc.vector.reduce_sum(idx2[:], tmp[:], axis=AX.X)
            # r = 1/(1+exp(l2-l1)) = gate of top-1; gate of top-2 = 1-r
            den = pp.tile([128, JSL], F32, tag="den")
            nc.vector.tensor_sub(den[:], l2[:], l1[:])
            nc.scalar.activation(den[:], den[:], ACT.Exp)
            nc.vector.tensor_scalar_add(den[:], den[:], 1.0)
            rden = pp.tile([128, JSL], F32, tag="rden")
            nc.vector.reciprocal(rden[:], den[:])
            # clamp r away from 1.0 so idx1 + r never rounds into idx1+1
            rcl = pp.tile([128, JSL], F32, tag="rcl")
            nc.vector.tensor_scalar_min(rcl[:], rden[:], 1.0 - 2.0 ** -12)
            pg = pp.tile([128, JSL, 2], F32, tag="pg")
            nc.vector.tensor_add(pg[:, :, 0], idx1[:], rcl[:])
            one1 = pp.tile([128, JSL], F32, tag="one1")
            nc.vector.memset(one1[:], 1.0)
            nc.vector.tensor_sub(one1[:], one1[:], rcl[:])  # gate2 = 1-r
            nc.vector.tensor_add(pg[:, :, 1], idx2[:], one1[:])
            # ship my slice, allgather the packed [NTOK, 2] gate matrix
            nc.sync.dma_start(
                ag_in[:].rearrange("(j p) e -> p j e", p=128), pg[:])
            nc.gpsimd.collective_compute(
                "AllGather", ALU.bypass, replica_groups=groups,
                ins=[ag_in[:]], outs=[ag_out[:]])

            # weights stream on SP behind the ag_in write: transfers run
            # during the AllGather window, done before the MLP needs them
            w1s = pp.tile([128, KD, H], BF16, tag="w1s")
            for k in range(KD):
                nc.sync.dma_start(w1s[:, k, :], w1[:, k, :])
            b1s = pp.tile([128, MH], F32, tag="b1s")
            nc.sync.dma_start(b1s[:], b1[:])
            w2s = pp.tile([128, KH, O], BF16, tag="w2s")
            for k in range(KH):
                nc.sync.dma_start(w2s[:, k, :], w2[:, k, :])
            b2s = pp.tile([1, O], BF16, tag="b2s")
            if with_b2:
                nc.sync.dma_start(b2s[:], b2[:])
                oness = pp.tile([1, 128], BF16, tag="oness")
                nc.sync.dma_start(oness[:], ones_c[:])

            ZR = min(max(1, 4096 // O), QT // 128)  # rows/partition per DMA
            zs = pp.tile([128, ZR * O], BF16, tag="zs")
            nc.vector.memset(zs[:], 0.0)

            # -------- my expert's routed tokens (all tokens) --------
            # partition-major token layout: t = p * JALL + j
            # (post-AG small DMAs ride the DVE queue: SP is busy with w2)
            agv = pp.tile([128, JALL, 2], F32, tag="agv")
            nc.scalar.dma_start(
                agv[:], ag_out[:].rearrange("(p j) e -> p j e", p=128))
            # mine = (e <= v < e+1); cand = v + (iota - e) since the
            # integer part of a matching v is exactly e
            iotme = pp.tile([128, JALL], F32, tag="iotme")
            eb0 = eids[:].broadcast_to([128, JALL])
            nc.vector.tensor_tensor(iotme[:], iotas[:], eb0, op=ALU.subtract)
            neg1 = pp.tile([128, JALL, 2], F32, tag="neg1")
            nc.vector.memset(neg1[:], -1.0)
            eb = eids[:].unsqueeze(-1).broadcast_to([128, JALL, 2])
            e1s = pp.tile([128, 1], F32, tag="e1s")
            nc.vector.tensor_scalar_add(e1s[:], eids[:], 1.0)
            e1b = e1s[:].unsqueeze(-1).broadcast_to([128, JALL, 2])
            mgeq = pp.tile([128, JALL, 2], mybir.dt.uint8, tag="mgeq")
            nc.vector.tensor_tensor(mgeq[:], agv[:], eb, op=ALU.is_ge)
            mlt = pp.tile([128, JALL, 2], mybir.dt.uint8, tag="mlt")
            nc.vector.tensor_tensor(mlt[:], agv[:], e1b, op=ALU.is_lt)
            m8 = pp.tile([128, JALL, 2], mybir.dt.uint8, tag="m8")
            nc.vector.tensor_tensor(m8[:], mgeq[:], mlt[:], op=ALU.mult)
            iob = iotme[:].unsqueeze(-1).broadcast_to([128, JALL, 2])
            cand = pp.tile([128, JALL, 2], F32, tag="cand")
            nc.vector.tensor_tensor(cand[:], agv[:], iob, op=ALU.add)
            sel = pp.tile([128, JALL, 2], F32, tag="sel")
            nc.vector.select(sel[:], m8[:], cand[:], neg1[:])
            mid = pp.tile([128, JALL], F32, tag="mid")
            nc.vector.reduce_max(mid[:], sel[:], axis=AX.X)

            # -------- per-quarter compaction + index prep --------
            # idx replication across the 8 gpsimd core groups is a PE
            # matmul against a block-identity selector (PE is idle here;
            # 8 small DMAs would cost ~8us of pipeline latency)
            FA = NQ * FCAP
            cmpa = pp.tile([16, FA], F32, tag="cmpa")
            gga = pp.tile([128, NQ, NTILE], F32, tag="gga")
            neg1q = pp.tile([16, FCAP], F32, tag="neg1q")
            nc.vector.memset(neg1q[:], -1.0)
            iga = [None] * NQ
            iss = [None] * NQ
            viq_dbg = [None] * NQ
            xg = [None] * NQ
            for q in range(NQ):
                mq = pp.tile([16, QT // 16], F32, tag=f"mq{q}")
                nc.scalar.dma_start(
                    mq[:].rearrange("p (a j) -> p a j", a=QP // 16),
                    mid[QP * q:QP * (q + 1), :])
                nf = pp.tile([1, 1], U32, tag=f"nf{q}")
                nc.gpsimd.sparse_gather(
                    cmpa[:, q * FCAP:(q + 1) * FCAP], mq[:],
                    num_found=nf[:])
                # the HW sparse_gather ucode writes garbage (even NaN)
                # past num_found: mask positions >= num_found to exact -1.
                # pos<nf is computed on one partition then relaid to the
                # [16, FCAP] col-major (pos = c*16 + r) layout.
                nff = pp.tile([1, 1], F32, tag=f"nff{q}")
                nc.vector.tensor_copy(nff[:], nf[:])
                nc.scalar.dma_start(nfd[q][:], nff[:])
                nfs = pp.tile([16, 1], F32, tag=f"nfs{q}")
                nc.scalar.dma_start(nfs[:], nfd[q][:].broadcast_to([16, 1]))
                m16 = pp.tile([16, FCAP], mybir.dt.uint8, tag=f"m16_{q}")
                nc.vector.tensor_tensor(
                    m16[:], poscs[:], nfs[:].broadcast_to([16, FCAP]),
                    op=ALU.is_lt)
                cmv = pp.tile([16, FCAP], F32, tag=f"cmv{q}")
                nc.vector.select(cmv[:], m16[:],
                                 cmpa[:, q * FCAP:(q + 1) * FCAP], neg1q[:])
                cq = cmv[:]
                # robust floor: f32->i16 convert may truncate (CoreSim) or
                # round (HW); correct by comparing the converted-back value
                i0 = pp.tile([16, FCAP], I16, tag=f"i0_{q}")
                nc.vector.tensor_copy(i0[:], cq)
                f0 = pp.tile([16, FCAP], F32, tag=f"f0_{q}")
                nc.vector.tensor_copy(f0[:], i0[:])
                up = pp.tile([16, FCAP], mybir.dt.uint8, tag=f"up{q}")
                nc.vector.tensor_tensor(up[:], f0[:], cq, op=ALU.is_gt)
                upi = pp.tile([16, FCAP], I16, tag=f"upi{q}")
                nc.vector.tensor_copy(upi[:], up[:])
                vi = pp.tile([16, FCAP], I16, tag=f"vi{q}")
                nc.vector.tensor_tensor(vi[:], i0[:], upi[:], op=ALU.subtract)
                flo = pp.tile([16, FCAP], F32, tag=f"flo{q}")
                nc.vector.tensor_copy(flo[:], vi[:])
                # gate fraction for this range (used by layer 2)
                frq = pp.tile([16, FCAP], F32, tag=f"frq{q}")
                nc.vector.tensor_sub(frq[:], cq, flo[:])
                gv = frq[:].rearrange("p (c g) -> p c g", g=8)
                for g8 in range(8):
                    nc.sync.dma_start(gga[g8 * 16:(g8 + 1) * 16, q, :],
                                      gv[:, :, g8])
                # replicate idx across the 8 gpsimd core groups via PE:
                # integer-valued f32 matmul is exact under bf16x2 on HW
                pr = psB.tile([128, FCAP], F32, tag="pr")
                nc.tensor.matmul(pr[:], sel16s[:], flo[:],
                                 start=True, stop=True)
                viq = pp.tile([128, FCAP], I16, tag=f"viq{q}")
                nc.vector.tensor_copy(viq[:], pr[:])
                viq_dbg[q] = viq
                # gather pads (-1) -> row 0 (garbage, discarded via trash)
                ig = pp.tile([128, FCAP], I16, tag=f"ig{q}")
                nc.vector.tensor_scalar_max(ig[:], viq[:], 0)
                iga[q] = ig
                # token gather (transposed into [d, slot]); one gather per
                # quarter at full QCAP capacity (transpose path needs
                # num_idxs%128==0); slots past SLOTS_Q gathered, not computed
                xgq = xp.tile([128, KD, QCAP], BF16, tag="xg")
                nc.gpsimd.dma_gather(
                    xgq[:], xbf[:], ig[:], QCAP, QCAP, D, transpose=True)
                xg[q] = xgq
                # scatter idx: quarter-local row, pads -> trash row QT
                # (off the gather critical path)
                loc = pp.tile([128, FCAP], I16, tag=f"loc{q}")
                nc.vector.tensor_scalar_add(loc[:], viq[:], -QT * q)
                lt = pp.tile([128, FCAP], mybir.dt.uint8, tag=f"lt{q}")
                nc.vector.tensor_single_scalar(lt[:], loc[:], 0, op=ALU.is_lt)
                tr = pp.tile([128, FCAP], I16, tag=f"tr{q}")
                nc.vector.tensor_scalar_mul(tr[:], viq[:], 0)
                nc.vector.tensor_scalar_add(tr[:], tr[:], QT)
                isq = pp.tile([128, FCAP], I16, tag=f"iss{q}")
                nc.vector.select(isq[:], lt[:], tr[:], loc[:])
                iss[q] = isq

            # ---- debug taps (DEBUG_DUMP builds only) ----
            if DEBUG_DUMP:
                dbg_stok = nc.dram_tensor("dbg_stok", [128, JSL, E], F32,
                                          kind="ExternalOutput")
                nc.sync.dma_start(dbg_stok[:], stok[:])
                dbg_pg = nc.dram_tensor("dbg_pg", [128, JSL, 2], F32,
                                        kind="ExternalOutput")
                nc.sync.dma_start(dbg_pg[:], pg[:])
                dbg_agv = nc.dram_tensor("dbg_agv", [128, JALL, 2], F32,
                                         kind="ExternalOutput")
                nc.sync.dma_start(dbg_agv[:], agv[:])
                dbg_mid = nc.dram_tensor("dbg_mid", [128, JALL], F32,
                                         kind="ExternalOutput")
                nc.sync.dma_start(dbg_mid[:], mid[:])
                dbg_cmpa = nc.dram_tensor("dbg_cmpa", [16, FA], F32,
                                          kind="ExternalOutput")
                nc.sync.dma_start(dbg_cmpa[:], cmpa[:])
                dbg_viq = nc.dram_tensor("dbg_viq", [128, FA], I16,
                                         kind="ExternalOutput")
                dbg_iga = nc.dram_tensor("dbg_iga", [128, FA], I16,
                                         kind="ExternalOutput")
                dbg_iss = nc.dram_tensor("dbg_iss", [128, FA], I16,
                                         kind="ExternalOutput")
                for q in range(NQ):
                    nc.sync.dma_start(
                        dbg_viq[:, q * FCAP:(q + 1) * FCAP], viq_dbg[q][:])
                    nc.sync.dma_start(
                        dbg_iga[:, q * FCAP:(q + 1) * FCAP], iga[q][:])
                    nc.sync.dma_start(
                        dbg_iss[:, q * FCAP:(q + 1) * FCAP], iss[q][:])
                dbg_gga = nc.dram_tensor("dbg_gga", [128, NQ, NTILE], F32,
                                         kind="ExternalOutput")
                nc.sync.dma_start(dbg_gga[:], gga[:])

            # zero the partial accumulators (row range the RS reads).
            # The dummy 0-multiply makes every zero-DMA depend on the last
            # gather, so the transfers enter the DMA queue after the
            # gathers: they fill the idle MLP window and are done well
            # before the first scatter-add needs them.
            nc.vector.tensor_scalar_mul(zs[0:1, 0:1],
                                        xg[NQ - 1][0:1, 0, 0:1], 0.0)
            for q in range(NQ):
                pv = parts[q][0:QT, :].rearrange("(a p z) o -> a p z o",
                                                 p=128, z=ZR)
                for a in range(QT // (128 * ZR)):
                    nc.sync.dma_start(
                        pv[a], zs[:].rearrange("p (z o) -> p z o", z=ZR))

            # -------- expert MLP + combine, one quarter at a time --------
            for q in range(NQ):
                hT = hp.tile([128, KH, SLOTS_Q], BF16, tag="hT")
                outg = ob.tile([128, NTILE, O], BF16, tag="outg")
                for off, csz in CH:
                    if csz % 128:  # unwritten tail partitions of last tile
                        nc.vector.memset(
                            outg[csz % 128:128, (off + csz) // 128, :], 0.0)
                for off, csz in CH:
                    for m in range(MH):
                        ph = psA.tile([128, 512], F32, tag="ph")
                        for k in range(KD):
                            nc.tensor.matmul(
                                ph[:, 0:csz], w1s[:, k, m * 128:(m + 1) * 128],
                                xg[q][:, k, off:off + csz],
                                start=(k == 0), stop=(k == KD - 1))
                        nc.scalar.activation(hT[:, m, off:off + csz],
                                             ph[:, 0:csz],
                                             ACT.Relu, bias=b1s[:, m:m + 1])
                    for mt in range((csz + 127) // 128):
                        sz = min(128, csz - mt * 128)
                        j = (off + mt * 128) // 128
                        for on in range(NO):
                            po = psC.tile([128, 512], F32, tag="po")
                            for k2 in range(KH):
                                nc.tensor.matmul(
                                    po[0:sz, :],
                                    hT[:, k2, off + mt * 128:
                                       off + mt * 128 + sz],
                                    w2s[:, k2, on * 512:(on + 1) * 512],
                                    start=(k2 == 0),
                                    stop=(not with_b2 and k2 == KH - 1))
                            if with_b2:
                                nc.tensor.matmul(
                                    po[0:sz, :], oness[:, 0:sz],
                                    b2s[0:1, on * 512:(on + 1) * 512],
                                    start=False, stop=True)
                            nc.vector.tensor_scalar_mul(
                                outg[0:sz, j, on * 512:(on + 1) * 512],
                                po[0:sz, :], gga[0:sz, q, j:j + 1])
                for off, csz in CH:
                    j0 = off // 128
                    nt = (csz + 127) // 128
                    cr = nt * 128  # pad num_idxs to 128; pads hit trash row
                    nc.gpsimd.dma_scatter_add(
                        parts[q][:], outg[:, j0:j0 + nt, :],
                        iss[q][:, off // 16:(off + cr) // 16],
                        cr, cr, O)
                nc.gpsimd.collective_compute(
                    "ReduceScatter", ALU.add, replica_groups=groups,
                    ins=[parts[q][0:QT, :]],
                    outs=[rs_out[SH * q:SH * (q + 1), :]])
                nc.sync.dma_start(y[SH * q:SH * (q + 1), :],
                                  rs_out[SH * q:SH * (q + 1), :])

    nc.compile()
    return nc


def make_in_maps(inputs, cfg=FULL):
    B, T, D, H, O, E = cfg["B"], cfg["T"], cfg["D"], cfg["H"], cfg["O"], cfg["E"]
    NTOK = B * T
    KD = D // 128
    KH = H // 128
    MH = H // 128
    TSL = NTOK // N_CORES

    x = np.ascontiguousarray(np.asarray(inputs["x"], dtype=np.float32)
                             .reshape(NTOK, D))
    gate_w = np.asarray(inputs["gate_w"], dtype=np.float32)
    gate_b = np.asarray(inputs["gate_b"], dtype=np.float32)
    w1 = np.asarray(inputs["w1"], dtype=np.float32)
    b1 = np.asarray(inputs["b1"], dtype=np.float32)
    w2 = np.asarray(inputs["w2"], dtype=np.float32)
    b2 = np.asarray(inputs["b2"], dtype=np.float32)
    assert int(inputs["num_experts_per_tok"]) == 2

    gw_p = np.ascontiguousarray(
        gate_w.reshape(KD, 128, E).transpose(1, 0, 2))
    gb_p = np.ascontiguousarray(gate_b.reshape(E, 1))
    xbf = np.ascontiguousarray(x.astype(ml_dtypes.bfloat16))

    maps = []
    for e in range(N_CORES):
        t0 = e * TSL
        xs = np.ascontiguousarray(
            x[t0:t0 + TSL, :].T.reshape(KD, 128, TSL).transpose(1, 0, 2))
        w1p = np.ascontiguousarray(
            w1[e].astype(ml_dtypes.bfloat16).reshape(KD, 128, H)
            .transpose(1, 0, 2))
        b1p = np.ascontiguousarray(b1[e].reshape(MH, 128).T)
        w2p = np.ascontiguousarray(
            w2[e].astype(ml_dtypes.bfloat16).reshape(KH, 128, O)
            .transpose(1, 0, 2))
        b2p = np.ascontiguousarray(
            b2[e].astype(ml_dtypes.bfloat16).reshape(1, O))
        eidp = np.full((128, 1), float(e), np.float32)
        maps.append({
            "xT": xs, "gw": gw_p, "gb": gb_p, "xbf": xbf,
            "w1": w1p, "b1": b1p, "w2": w2p, "b2": b2p, "eid": eidp,
        })
    return maps


def unshard_y(ys, cfg=FULL):
    """ys[i] is core i's [TSL, O]; quarter q of core i holds tokens
    [QT*q + SH*i, QT*q + SH*(i+1))."""
    B, T, O = cfg["B"], cfg["T"], cfg["O"]
    NTOK = B * T
    TSL = NTOK // N_CORES
    QT = NTOK // NQ
    SH = TSL // NQ
    out = np.empty((NTOK, O), np.float32)
    for i in range(N_CORES):
        yi = np.asarray(ys[i]).astype(np.float32)
        for q in range(NQ):
            out[QT * q + SH * i: QT * q + SH * (i + 1)] = \
                yi[SH * q: SH * (q + 1)]
    return out.reshape(B, T, O)


def _routing_max_quarter(inputs, cfg=FULL):
    """Host-side routing census: max routed count per (expert, quarter).
    Used only to pick a safe SLOTS_Q; ~30ms of numpy."""
    x = np.asarray(inputs["x"], np.float32).reshape(-1, cfg["D"])
    logits = (x @ np.asarray(inputs["gate_w"], np.float32)
              + np.asarray(inputs["gate_b"], np.float32))
    top2 = np.argpartition(-logits, 1, axis=-1)[:, :2]
    NTOK = x.shape[0]
    QT = NTOK // NQ
    qidx = np.arange(NTOK) // QT
    cnt = np.zeros((cfg["E"], NQ), np.int64)
    np.add.at(cnt, (top2[:, 0], qidx), 1)
    np.add.at(cnt, (top2[:, 1], qidx), 1)
    return int(cnt.max())


_NC_CACHE = {}


def kernel(**inputs) -> np.ndarray:
    import time as _time
    cfg = dict(FULL)
    maps = make_in_maps(inputs, cfg)
    need_b2 = bool(np.any(np.asarray(inputs["b2"], dtype=np.float32)))
    maxq = _routing_max_quarter(inputs, cfg)
    if maxq > cfg["SLOTS_Q"]:  # input drift: rebuild with enough capacity
        cfg["SLOTS_Q"] = ((maxq + 16 + 63) // 64) * 64
    key = (need_b2, cfg["SLOTS_Q"])
    last_err = None
    for attempt in range(4):
        try:
            if _NC_CACHE.get("key") != key:
                _NC_CACHE.clear()
                _NC_CACHE["nc"] = build(cfg, with_b2=need_b2)
                _NC_CACHE["key"] = key
            res = run_bass_kernel_spmd(
                _NC_CACHE["nc"], maps, core_ids=list(range(N_CORES)))
            ys = [np.asarray(res.results[i]["y"]) for i in range(N_CORES)]
            out = unshard_y(ys, cfg)
            # a wedged device can "succeed" with garbage; legitimate outputs
            # for this problem have absmax of a few units
            if not np.isfinite(out).all() or np.abs(out).max() > 1e3:
                raise RuntimeError(
                    f"implausible output (absmax={np.abs(out).max()}), "
                    "retrying on a rebuilt kernel")
            return out
        except Exception as e:  # device wedge / transient runtime failure
            last_err = e
            _NC_CACHE.clear()
            _time.sleep(20 * (attempt + 1))
    raise last_err


# revision 53
# speedup vs baseline: 1.0146x; 1.0146x over previous
"""Expert-parallel MoE kernel for Trainium2 (8 NeuronCores), v2.

Reference computation (dense in the reference, but top-2 sparse in effect):
  scores = softmax(x @ gate_w + gate_b)          [B,T,E]
  keep top-2 per token, L1-renormalize -> g      [B,T,E] (only 2 nonzero)
  out = sum_e g[:,e] * (relu(x@w1[e]+b1[e]) @ w2[e] + b2[e])

Strategy (all compute on device):
  - Core e owns expert e (weights sharded along E).
  - Gating is token-sharded: core i computes full top-2 gates for tokens
    [i*NTOK/8, (i+1)*NTOK/8) in fp32, packs each token's two picks as
    (expert_idx + gate_fraction) into a [NTOK, 2] f32 AllGather payload.
  - Tokens are processed in 4 "quarters" (token ranges of NTOK/4). Each
    core compacts the token ids routed to its expert per quarter
    (sparse_gather of iota+gate packed values), gathers those token rows
    of x (bf16) with a transposing dma_gather, runs the two matmuls in
    bf16 (fp32 accumulation), scales by the gate values and scatter-adds
    into a per-quarter zeroed [QT, O] partial (bf16, zeroed by DMAs that
    are dependency-gated to run during the MLP window).
  - Four ReduceScatters (one per quarter, issued as soon as that
    quarter's scatter-adds land) sum the partials across cores and write
    directly into interleaved blocks of the y output; the host
    reassembles the block permutation. Only the last RS sits on the
    critical path.
"""

import numpy as np
import ml_dtypes

import concourse.bacc as bacc
import concourse.bass as bass
import concourse.mybir as mybir
import concourse.tile as tile
from concourse.bass_utils import run_bass_kernel_spmd

F32 = mybir.dt.float32
BF16 = mybir.dt.bfloat16
I16 = mybir.dt.int16
U32 = mybir.dt.uint32
AX = mybir.AxisListType
ALU = mybir.AluOpType
ACT = mybir.ActivationFunctionType

# Full-problem constants (hardcoded per the harness contract).
# SLOTS_Q: computed capacity per token-quarter (max routed count for the
# fixed seed-0 inputs is 559 per (expert, quarter); margin 17).
FULL = dict(B=4, T=2048, D=1024, H=2048, O=1024, E=8, SLOTS_Q=576)
N_CORES = 8
NQ = 4  # token quarters


def _chunks(slots):
    out = []
    off = 0
    while off < slots:
        c = min(512, slots - off)
        out.append((off, c))
        off += c
    return out


DEBUG_DUMP = False


def build(cfg=FULL, with_b2=False, warmup=4):
    B, T, D, H, O, E = cfg["B"], cfg["T"], cfg["D"], cfg["H"], cfg["O"], cfg["E"]
    SLOTS_Q = cfg["SLOTS_Q"]
    NTOK = B * T
    KD = D // 128           # K-tiles in D
    KH = H // 128           # K-tiles in H
    MH = H // 128           # M-tiles for layer 1
    NO = O // 512           # 512-wide N-chunks in O for layer 2
    TSL = NTOK // N_CORES   # gating token slice per core
    JSL = TSL // 128        # token tiles in my gating slice
    JALL = NTOK // 128      # free-dim length of token-major [128, JALL] tiles
    QT = NTOK // NQ         # tokens per quarter
    QP = 128 // NQ          # partitions per quarter in [128, JALL] layout
    SH = TSL // NQ          # per-core y rows per quarter (RS shard)
    NTILE = (SLOTS_Q + 127) // 128   # 128-slot tiles per quarter
    QCAP = 128 * NTILE               # compaction array capacity
    FCAP = QCAP // 16                # sparse_gather output cols
    CH = _chunks(SLOTS_Q)
    assert FCAP % 8 == 0 and QT % 16 == 0 and TSL % NQ == 0

    nc = bacc.Bacc("TRN2", target_bir_lowering=False, debug=False,
                   num_devices=N_CORES)

    # ---- I/O ----
    xT = nc.dram_tensor("xT", [128, KD, TSL], F32, kind="ExternalInput")
    gw = nc.dram_tensor("gw", [128, KD, E], F32, kind="ExternalInput")
    gb = nc.dram_tensor("gb", [E, 1], F32, kind="ExternalInput")
    xbf = nc.dram_tensor("xbf", [NTOK, D], BF16, kind="ExternalInput")
    w1 = nc.dram_tensor("w1", [128, KD, H], BF16, kind="ExternalInput")
    b1 = nc.dram_tensor("b1", [128, MH], F32, kind="ExternalInput")
    w2 = nc.dram_tensor("w2", [128, KH, O], BF16, kind="ExternalInput")
    b2 = nc.dram_tensor("b2", [1, O], BF16, kind="ExternalInput")
    eid = nc.dram_tensor("eid", [128, 1], F32, kind="ExternalInput")
    y = nc.dram_tensor("y", [TSL, O], BF16, kind="ExternalOutput")
    # Per-quarter partial accumulators (internal: NeuronCC forbids
    # collectives reading IO tensors). Rows [0, QT) are zeroed on device
    # off the critical path; +128 trash rows take the capacity-padding
    # scatter-adds and are never read.
    parts = [nc.dram_tensor(f"partial{q}", [QT + 128, O], BF16)
             for q in range(NQ)]

    # ---- constants (embedded in NEFF) ----
    # token-major iota for the partition-major post-AG layout:
    # token t = p * JALL + j
    iota_np = (np.arange(128)[:, None] * JALL
               + np.arange(JALL)[None, :]).astype(np.float32)
    iota_c = nc.inline_tensor(iota_np, name="iota_c")
    iota8_np = np.tile(np.arange(E, dtype=np.float32), (128, 1))
    iota8_c = nc.inline_tensor(iota8_np, name="iota8_c")
    id8_c = nc.inline_tensor(np.eye(E, dtype=np.float32), name="id8_c")
    ones_c = nc.inline_tensor(np.ones((1, 128), dtype=ml_dtypes.bfloat16),
                              name="ones_c")
    sel16_np = np.tile(np.eye(16, dtype=np.float32), (1, 8))
    sel16_c = nc.inline_tensor(sel16_np, name="sel16_c")
    onesf_c = nc.inline_tensor(np.ones((1, 128), np.float32), name="onesf_c")
    pos128_np = (np.arange(FCAP)[None, :] * 16
                 + (np.arange(128) % 16)[:, None]).astype(np.float32)
    pos128_c = nc.inline_tensor(pos128_np, name="pos128_c")
    neg1i_c = nc.inline_tensor(np.full((128, FCAP), -1, np.int16),
                               name="neg1i_c")

    # ---- internal DRAM (collective operands) ----
    ag_in = nc.dram_tensor("ag_in", [TSL, 2], F32)
    ag_out = nc.dram_tensor("ag_out", [NTOK, 2], F32)
    rs_out = nc.dram_tensor("rs_out", [TSL, O], BF16)

    groups = [list(range(N_CORES))]

    with tile.TileContext(nc) as tc:
        with (
            tc.tile_pool(name="persist", bufs=1) as pp,
            tc.tile_pool(name="stream", bufs=5) as sp,
            tc.tile_pool(name="xgp", bufs=4) as xp,
            tc.tile_pool(name="hbuf", bufs=2) as hp,
            tc.tile_pool(name="outb", bufs=2) as ob,
            tc.tile_pool(name="psA", bufs=2, space="PSUM") as psA,
            tc.tile_pool(name="psG", bufs=2, space="PSUM") as psG,
            tc.tile_pool(name="psB", bufs=1, space="PSUM") as psB,
            tc.tile_pool(name="psC", bufs=2, space="PSUM") as psC,
        ):
            # ---- latency-critical consts for gating (SP queue) ----
            gws = pp.tile([128, KD, E], F32, tag="gws")
            nc.sync.dma_start(gws[:], gw[:])
            gbs = pp.tile([E, 1], F32, tag="gbs")
            nc.sync.dma_start(gbs[:], gb[:])
            id8s = pp.tile([E, E], F32, tag="id8s")
            nc.sync.dma_start(id8s[:], id8_c[:])

            GC = min(512, TSL)

            # ---- PE warmup: garbage matmuls to climb the p-state ramp
            # while the gating activations stream in ----
            if warmup:
                wt = pp.tile([128, GC], BF16, tag="warm")
                nc.vector.memset(wt[:], 0.0)
                for _ in range(warmup):
                    pw = psG.tile([E, GC], F32, tag="ps_gate")
                    nc.tensor.matmul(pw[:], wt[:, 0:E], wt[:],
                                     start=True, stop=True)

            # ---------- gating for my token slice (fp32) ----------
            stok = pp.tile([128, JSL, E], F32, tag="stok")
            for nch2 in range(TSL // GC):
                ps = psG.tile([E, GC], F32, tag="ps_gate")
                for k in range(KD):
                    xk = sp.tile([128, GC], F32, tag="xk")
                    nc.sync.dma_start(
                        xk[:], xT[:, k, nch2 * GC:(nch2 + 1) * GC])
                    nc.tensor.matmul(ps[:], gws[:, k, :], xk[:],
                                     start=(k == 0), stop=(k == KD - 1))
                sct = sp.tile([E, GC], F32, tag="sct")
                nc.vector.tensor_scalar_add(sct[:], ps[:], gbs[:])
                for tt in range(GC // 128):
                    pst = psB.tile([128, E], F32, tag="pst")
                    nc.tensor.matmul(
                        pst[:], sct[:, tt * 128:(tt + 1) * 128], id8s[:],
                        start=True, stop=True)
                    nc.vector.tensor_copy(
                        stok[:, nch2 * (GC // 128) + tt, :], pst[:])

            # consts used after gating (emitted behind the xk loads on SP)
            iota8s = pp.tile([128, E], F32, tag="iota8s")
            nc.sync.dma_start(iota8s[:], iota8_c[:])
            eids = pp.tile([128, 1], F32, tag="eids")
            nc.sync.dma_start(eids[:], eid[:])
            iotas = pp.tile([128, JALL], F32, tag="iotas")
            nc.sync.dma_start(iotas[:], iota_c[:])
            sel16s = pp.tile([16, 128], F32, tag="sel16s")
            nc.sync.dma_start(sel16s[:], sel16_c[:])
            onesfs = pp.tile([1, 128], F32, tag="onesfs")
            nc.sync.dma_start(onesfs[:], onesf_c[:])
            pos128s = pp.tile([128, FCAP], F32, tag="pos128s")
            nc.sync.dma_start(pos128s[:], pos128_c[:])
            neg1is = pp.tile([128, FCAP], I16, tag="neg1is")
            nc.sync.dma_start(neg1is[:], neg1i_c[:])

            # -------- top-2 + packed (idx + gate) for my slice --------
            l1 = pp.tile([128, JSL], F32, tag="l1")
            nc.vector.reduce_max(l1[:], stok[:], axis=AX.X)
            l1b = l1[:].unsqueeze(-1).broadcast_to([128, JSL, E])
            eq = pp.tile([128, JSL, E], F32, tag="eq")
            nc.vector.tensor_tensor(eq[:], stok[:], l1b, op=ALU.is_equal)
            msc = pp.tile([128, JSL, E], F32, tag="msc")
            nc.vector.tensor_scalar_mul(msc[:], eq[:], -1e30)
            nc.vector.tensor_add(msc[:], msc[:], stok[:])  # masked scores
            l2 = pp.tile([128, JSL], F32, tag="l2")
            nc.vector.reduce_max(l2[:], msc[:], axis=AX.X)
            l2b = l2[:].unsqueeze(-1).broadcast_to([128, JSL, E])
            # idx1 = argmax, idx2 = arg-2nd-max via iota dot products
            i8b = iota8s[:].unsqueeze(1).broadcast_to([128, JSL, E])
            tmp = pp.tile([128, JSL, E], F32, tag="tmp")
            nc.vector.tensor_tensor(tmp[:], eq[:], i8b, op=ALU.mult)
            idx1 = pp.tile([128, JSL], F32, tag="idx1")
            nc.vector.reduce_sum(idx1[:], tmp[:], axis=AX.X)
            msk2 = pp.tile([128, JSL, E], F32, tag="msk2")
            nc.vector.tensor_tensor(msk2[:], stok[:], l2b, op=ALU.is_ge)
            nc.vector.tensor_sub(msk2[:], msk2[:], eq[:])
            nc.vector.tensor_tensor(tmp[:], msk2[:], i8b, op=ALU.mult)
            idx2 = pp.tile([128, JSL], F32, tag="idx2")
            nc.vector.reduce_sum(idx2[:], tmp[:], axis=AX.X)
            # r = 1/(1+exp(l2-l1)) = gate of top-1; gate of top-2 = 1-r
            den = pp.tile([128, JSL], F32, tag="den")
            nc.vector.tensor_sub(den[:], l2[:], l1[:])
            nc.scalar.activation(den[:], den[:], ACT.Exp)
            nc.vector.tensor_scalar_add(den[:], den[:], 1.0)
            rden = pp.tile([128, JSL], F32, tag="rden")
            nc.vector.reciprocal(rden[:], den[:])
            # clamp r away from 1.0 so idx1 + r never rounds into idx1+1
            rcl = pp.tile([128, JSL], F32, tag="rcl")
            nc.vector.tensor_scalar_min(rcl[:], rden[:], 1.0 - 2.0 ** -12)
            pg = pp.tile([128, JSL, 2], F32, tag="pg")
            nc.vector.tensor_add(pg[:, :, 0], idx1[:], rcl[:])
            one1 = pp.tile([128, JSL], F32, tag="one1")
            nc.vector.memset(one1[:], 1.0)
            nc.vector.tensor_sub(one1[:], one1[:], rcl[:])  # gate2 = 1-r
            nc.vector.tensor_add(pg[:, :, 1], idx2[:], one1[:])
            # ship my slice, allgather the packed [NTOK, 2] gate matrix
            nc.sync.dma_start(
                ag_in[:].rearrange("(j p) e -> p j e", p=128), pg[:])
            nc.gpsimd.collective_compute(
                "AllGather", ALU.bypass, replica_groups=groups,
                ins=[ag_in[:]], outs=[ag_out[:]])

            # weights stream on SP behind the ag_in write: transfers run
            # during the AllGather window, done before the MLP needs them
            w1s = pp.tile([128, KD, H], BF16, tag="w1s")
            for k in range(KD):
                nc.sync.dma_start(w1s[:, k, :], w1[:, k, :])
            b1s = pp.tile([128, MH], F32, tag="b1s")
            nc.sync.dma_start(b1s[:], b1[:])
            w2s = pp.tile([128, KH, O], BF16, tag="w2s")
            for k in range(KH):
                nc.sync.dma_start(w2s[:, k, :], w2[:, k, :])
            b2s = pp.tile([1, O], BF16, tag="b2s")
            if with_b2:
                nc.sync.dma_start(b2s[:], b2[:])
                oness = pp.tile([1, 128], BF16, tag="oness")
                nc.sync.dma_start(oness[:], ones_c[:])

            ZR = min(max(1, 4096 // O), QT // 128)  # rows/partition per DMA
            zs = pp.tile([128, ZR * O], BF16, tag="zs")
            nc.vector.memset(zs[:], 0.0)

            # -------- my expert's routed tokens (all tokens) --------
            # partition-major token layout: t = p * JALL + j
            # (post-AG small DMAs ride the DVE queue: SP is busy with w2)
            agv = pp.tile([128, JALL, 2], F32, tag="agv")
            nc.scalar.dma_start(
                agv[:], ag_out[:].rearrange("(p j) e -> p j e", p=128))
            # mine = (e <= v < e+1); cand = v + (iota - e) since the
            # integer part of a matching v is exactly e
            iotme = pp.tile([128, JALL], F32, tag="iotme")
            eb0 = eids[:].broadcast_to([128, JALL])
            nc.vector.tensor_tensor(iotme[:], iotas[:], eb0, op=ALU.subtract)
            neg1 = pp.tile([128, JALL, 2], F32, tag="neg1")
            nc.vector.memset(neg1[:], -1.0)
            eb = eids[:].unsqueeze(-1).broadcast_to([128, JALL, 2])
            e1s = pp.tile([128, 1], F32, tag="e1s")
            nc.vector.tensor_scalar_add(e1s[:], eids[:], 1.0)
            e1b = e1s[:].unsqueeze(-1).broadcast_to([128, JALL, 2])
            mgeq = pp.tile([128, JALL, 2], mybir.dt.uint8, tag="mgeq")
            nc.vector.tensor_tensor(mgeq[:], agv[:], eb, op=ALU.is_ge)
            mlt = pp.tile([128, JALL, 2], mybir.dt.uint8, tag="mlt")
            nc.vector.tensor_tensor(mlt[:], agv[:], e1b, op=ALU.is_lt)
            m8 = pp.tile([128, JALL, 2], mybir.dt.uint8, tag="m8")
            nc.vector.tensor_tensor(m8[:], mgeq[:], mlt[:], op=ALU.mult)
            iob = iotme[:].unsqueeze(-1).broadcast_to([128, JALL, 2])
            cand = pp.tile([128, JALL, 2], F32, tag="cand")
            nc.vector.tensor_tensor(cand[:], agv[:], iob, op=ALU.add)
            sel = pp.tile([128, JALL, 2], F32, tag="sel")
            nc.vector.select(sel[:], m8[:], cand[:], neg1[:])
            mid = pp.tile([128, JALL], F32, tag="mid")
            nc.vector.reduce_max(mid[:], sel[:], axis=AX.X)

            # -------- per-quarter compaction + index prep --------
            # idx replication across the 8 gpsimd core groups is a PE
            # matmul against a block-identity selector (PE is idle here;
            # 8 small DMAs would cost ~8us of pipeline latency)
            FA = NQ * FCAP
            cmpa = pp.tile([16, FA], F32, tag="cmpa")
            gga = pp.tile([128, NQ, NTILE], F32, tag="gga")
            iga = [None] * NQ
            iss = [None] * NQ
            viq_dbg = [None] * NQ
            xg = [None] * NQ
            for q in range(NQ):
                mq = pp.tile([16, QT // 16], F32, tag=f"mq{q}")
                nc.scalar.dma_start(
                    mq[:].rearrange("p (a j) -> p a j", a=QP // 16),
                    mid[QP * q:QP * (q + 1), :])
                nf = pp.tile([1, 1], U32, tag=f"nf{q}")
                nc.gpsimd.sparse_gather(
                    cmpa[:, q * FCAP:(q + 1) * FCAP], mq[:],
                    num_found=nf[:])
                # HW sparse_gather writes garbage (even NaN) past
                # num_found. Only the scatter/gather INDICES must be
                # sanitized (garbage gates flow to the trash row): the
                # mask is applied to the replicated idx below, with
                # num_found broadcast to 128 partitions by a K=1
                # ones-matmul (exact: integer-valued f32).
                nff = pp.tile([1, 1], F32, tag=f"nff{q}")
                nc.vector.tensor_copy(nff[:], nf[:])
                pn = psB.tile([128, E], F32, tag="pst")
                nc.tensor.matmul(pn[:, 0:1], onesfs[:], nff[:],
                                 start=True, stop=True)
                nf128 = pp.tile([128, 1], F32, tag=f"nf128_{q}")
                nc.vector.tensor_copy(nf128[:], pn[:, 0:1])
                m128 = pp.tile([128, FCAP], mybir.dt.uint8, tag=f"m128_{q}")
                nc.vector.tensor_tensor(
                    m128[:], pos128s[:], nf128[:].broadcast_to([128, FCAP]),
                    op=ALU.is_lt)
                cq = cmpa[:, q * FCAP:(q + 1) * FCAP]
                # robust floor: f32->i16 convert may truncate (CoreSim) or
                # round (HW); correct by comparing the converted-back value
                i0 = pp.tile([16, FCAP], I16, tag=f"i0_{q}")
                nc.vector.tensor_copy(i0[:], cq)
                f0 = pp.tile([16, FCAP], F32, tag=f"f0_{q}")
                nc.vector.tensor_copy(f0[:], i0[:])
                up = pp.tile([16, FCAP], mybir.dt.uint8, tag=f"up{q}")
                nc.vector.tensor_tensor(up[:], f0[:], cq, op=ALU.is_gt)
                upi = pp.tile([16, FCAP], I16, tag=f"upi{q}")
                nc.vector.tensor_copy(upi[:], up[:])
                vi = pp.tile([16, FCAP], I16, tag=f"vi{q}")
                nc.vector.tensor_tensor(vi[:], i0[:], upi[:], op=ALU.subtract)
                flo = pp.tile([16, FCAP], F32, tag=f"flo{q}")
                nc.vector.tensor_copy(flo[:], vi[:])
                # gate fraction for this range (used by layer 2)
                frq = pp.tile([16, FCAP], F32, tag=f"frq{q}")
                nc.vector.tensor_sub(frq[:], cq, flo[:])
                gv = frq[:].rearrange("p (c g) -> p c g", g=8)
                for g8 in range(8):
                    nc.sync.dma_start(gga[g8 * 16:(g8 + 1) * 16, q, :],
                                      gv[:, :, g8])
                # replicate idx across the 8 gpsimd core groups via PE:
                # integer-valued f32 matmul is exact under bf16x2 on HW
                pr = psB.tile([128, FCAP], F32, tag="pr")
                nc.tensor.matmul(pr[:], sel16s[:], flo[:],
                                 start=True, stop=True)
                viq0 = pp.tile([128, FCAP], I16, tag=f"viq0_{q}")
                nc.vector.tensor_copy(viq0[:], pr[:])
                viq = pp.tile([128, FCAP], I16, tag=f"viq{q}")
                nc.vector.select(viq[:], m128[:], viq0[:], neg1is[:])
                viq_dbg[q] = viq
                # gather pads (-1) -> row 0 (garbage, discarded via trash)
                ig = pp.tile([128, FCAP], I16, tag=f"ig{q}")
                nc.vector.tensor_scalar_max(ig[:], viq[:], 0)
                iga[q] = ig
                # token gather (transposed into [d, slot]); one gather per
                # quarter at full QCAP capacity (transpose path needs
                # num_idxs%128==0); slots past SLOTS_Q gathered, not computed
                xgq = xp.tile([128, KD, QCAP], BF16, tag="xg")
                nc.gpsimd.dma_gather(
                    xgq[:], xbf[:], ig[:], QCAP, QCAP, D, transpose=True)
                xg[q] = xgq
                # scatter idx: quarter-local row, pads -> trash row QT
                # (off the gather critical path)
                loc = pp.tile([128, FCAP], I16, tag=f"loc{q}")
                nc.vector.tensor_scalar_add(loc[:], viq[:], -QT * q)
                lt = pp.tile([128, FCAP], mybir.dt.uint8, tag=f"lt{q}")
                nc.vector.tensor_single_scalar(lt[:], loc[:], 0, op=ALU.is_lt)
                tr = pp.tile([128, FCAP], I16, tag=f"tr{q}")
                nc.vector.tensor_scalar_mul(tr[:], viq[:], 0)
                nc.vector.tensor_scalar_add(tr[:], tr[:], QT)
                isq = pp.tile([128, FCAP], I16, tag=f"iss{q}")
                nc.vector.select(isq[:], lt[:], tr[:], loc[:])
                iss[q] = isq

            # ---- debug taps (DEBUG_DUMP builds only) ----
            if DEBUG_DUMP:
                dbg_stok = nc.dram_tensor("dbg_stok", [128, JSL, E], F32,
                                          kind="ExternalOutput")
                nc.sync.dma_start(dbg_stok[:], stok[:])
                dbg_pg = nc.dram_tensor("dbg_pg", [128, JSL, 2], F32,
                                        kind="ExternalOutput")
                nc.sync.dma_start(dbg_pg[:], pg[:])
                dbg_agv = nc.dram_tensor("dbg_agv", [128, JALL, 2], F32,
                                         kind="ExternalOutput")
                nc.sync.dma_start(dbg_agv[:], agv[:])
                dbg_mid = nc.dram_tensor("dbg_mid", [128, JALL], F32,
                                         kind="ExternalOutput")
                nc.sync.dma_start(dbg_mid[:], mid[:])
                dbg_cmpa = nc.dram_tensor("dbg_cmpa", [16, FA], F32,
                                          kind="ExternalOutput")
                nc.sync.dma_start(dbg_cmpa[:], cmpa[:])
                dbg_viq = nc.dram_tensor("dbg_viq", [128, FA], I16,
                                         kind="ExternalOutput")
                dbg_iga = nc.dram_tensor("dbg_iga", [128, FA], I16,
                                         kind="ExternalOutput")
                dbg_iss = nc.dram_tensor("dbg_iss", [128, FA], I16,
                                         kind="ExternalOutput")
                for q in range(NQ):
                    nc.sync.dma_start(
                        dbg_viq[:, q * FCAP:(q + 1) * FCAP], viq_dbg[q][:])
                    nc.sync.dma_start(
                        dbg_iga[:, q * FCAP:(q + 1) * FCAP], iga[q][:])
                    nc.sync.dma_start(
                        dbg_iss[:, q * FCAP:(q + 1) * FCAP], iss[q][:])
                dbg_gga = nc.dram_tensor("dbg_gga", [128, NQ, NTILE], F32,
                                         kind="ExternalOutput")
                nc.sync.dma_start(dbg_gga[:], gga[:])

            # zero the partial accumulators (row range the RS reads).
            # The dummy 0-multiply makes every zero-DMA depend on the last
            # gather, so the transfers enter the DMA queue after the
            # gathers: they fill the idle MLP window and are done well
            # before the first scatter-add needs them.
            nc.vector.tensor_scalar_mul(zs[0:1, 0:1],
                                        xg[NQ - 1][0:1, 0, 0:1], 0.0)
            for q in range(NQ):
                pv = parts[q][0:QT, :].rearrange("(a p z) o -> a p z o",
                                                 p=128, z=ZR)
                for a in range(QT // (128 * ZR)):
                    nc.sync.dma_start(
                        pv[a], zs[:].rearrange("p (z o) -> p z o", z=ZR))

            # -------- expert MLP + combine, one quarter at a time --------
            for q in range(NQ):
                hT = hp.tile([128, KH, SLOTS_Q], BF16, tag="hT")
                outg = ob.tile([128, NTILE, O], BF16, tag="outg")
                for off, csz in CH:
                    if csz % 128:  # unwritten tail partitions of last tile
                        nc.vector.memset(
                            outg[csz % 128:128, (off + csz) // 128, :], 0.0)
                for off, csz in CH:
                    for m in range(MH):
                        ph = psA.tile([128, 512], F32, tag="ph")
                        for k in range(KD):
                            nc.tensor.matmul(
                                ph[:, 0:csz], w1s[:, k, m * 128:(m + 1) * 128],
                                xg[q][:, k, off:off + csz],
                                start=(k == 0), stop=(k == KD - 1))
                        nc.scalar.activation(hT[:, m, off:off + csz],
                                             ph[:, 0:csz],
                                             ACT.Relu, bias=b1s[:, m:m + 1])
                    for mt in range((csz + 127) // 128):
                        sz = min(128, csz - mt * 128)
                        j = (off + mt * 128) // 128
                        for on in range(NO):
                            po = psC.tile([128, 512], F32, tag="po")
                            for k2 in range(KH):
                                nc.tensor.matmul(
                                    po[0:sz, :],
                                    hT[:, k2, off + mt * 128:
                                       off + mt * 128 + sz],
                                    w2s[:, k2, on * 512:(on + 1) * 512],
                                    start=(k2 == 0),
                                    stop=(not with_b2 and k2 == KH - 1))
                            if with_b2:
                                nc.tensor.matmul(
                                    po[0:sz, :], oness[:, 0:sz],
                                    b2s[0:1, on * 512:(on + 1) * 512],
                                    start=False, stop=True)
                            nc.vector.tensor_scalar_mul(
                                outg[0:sz, j, on * 512:(on + 1) * 512],
                                po[0:sz, :], gga[0:sz, q, j:j + 1])
                for off, csz in CH:
                    j0 = off // 128
                    nt = (csz + 127) // 128
                    cr = nt * 128  # pad num_idxs to 128; pads hit trash row
                    nc.gpsimd.dma_scatter_add(
                        parts[q][:], outg[:, j0:j0 + nt, :],
                        iss[q][:, off // 16:(off + cr) // 16],
                        cr, cr, O)
                nc.gpsimd.collective_compute(
                    "ReduceScatter", ALU.add, replica_groups=groups,
                    ins=[parts[q][0:QT, :]],
                    outs=[rs_out[SH * q:SH * (q + 1), :]])
                nc.sync.dma_start(y[SH * q:SH * (q + 1), :],
                                  rs_out[SH * q:SH * (q + 1), :])

    nc.compile()
    return nc


def make_in_maps(inputs, cfg=FULL):
    B, T, D, H, O, E = cfg["B"], cfg["T"], cfg["D"], cfg["H"], cfg["O"], cfg["E"]
    NTOK = B * T
    KD = D // 128
    KH = H // 128
    MH = H // 128
    TSL = NTOK // N_CORES

    x = np.ascontiguousarray(np.asarray(inputs["x"], dtype=np.float32)
                             .reshape(NTOK, D))
    gate_w = np.asarray(inputs["gate_w"], dtype=np.float32)
    gate_b = np.asarray(inputs["gate_b"], dtype=np.float32)
    w1 = np.asarray(inputs["w1"], dtype=np.float32)
    b1 = np.asarray(inputs["b1"], dtype=np.float32)
    w2 = np.asarray(inputs["w2"], dtype=np.float32)
    b2 = np.asarray(inputs["b2"], dtype=np.float32)
    assert int(inputs["num_experts_per_tok"]) == 2

    gw_p = np.ascontiguousarray(
        gate_w.reshape(KD, 128, E).transpose(1, 0, 2))
    gb_p = np.ascontiguousarray(gate_b.reshape(E, 1))
    xbf = np.ascontiguousarray(x.astype(ml_dtypes.bfloat16))

    maps = []
    for e in range(N_CORES):
        t0 = e * TSL
        xs = np.ascontiguousarray(
            x[t0:t0 + TSL, :].T.reshape(KD, 128, TSL).transpose(1, 0, 2))
        w1p = np.ascontiguousarray(
            w1[e].astype(ml_dtypes.bfloat16).reshape(KD, 128, H)
            .transpose(1, 0, 2))
        b1p = np.ascontiguousarray(b1[e].reshape(MH, 128).T)
        w2p = np.ascontiguousarray(
            w2[e].astype(ml_dtypes.bfloat16).reshape(KH, 128, O)
            .transpose(1, 0, 2))
        b2p = np.ascontiguousarray(
            b2[e].astype(ml_dtypes.bfloat16).reshape(1, O))
        eidp = np.full((128, 1), float(e), np.float32)
        maps.append({
            "xT": xs, "gw": gw_p, "gb": gb_p, "xbf": xbf,
            "w1": w1p, "b1": b1p, "w2": w2p, "b2": b2p, "eid": eidp,
        })
    return maps


def unshard_y(ys, cfg=FULL):
    """ys[i] is core i's [TSL, O]; quarter q of core i holds tokens
    [QT*q + SH*i, QT*q + SH*(i+1))."""
    B, T, O = cfg["B"], cfg["T"], cfg["O"]
    NTOK = B * T
    TSL = NTOK // N_CORES
    QT = NTOK // NQ
    SH = TSL // NQ
    out = np.empty((NTOK, O), np.float32)
    for i in range(N_CORES):
        yi = np.asarray(ys[i]).astype(np.float32)
        for q in range(NQ):
            out[QT * q + SH * i: QT * q + SH * (i + 1)] = \
                yi[SH * q: SH * (q + 1)]
    return out.reshape(B, T, O)


def _routing_max_quarter(inputs, cfg=FULL):
    """Host-side routing census: max routed count per (expert, quarter).
    Used only to pick a safe SLOTS_Q; ~30ms of numpy."""
    x = np.asarray(inputs["x"], np.float32).reshape(-1, cfg["D"])
    logits = (x @ np.asarray(inputs["gate_w"], np.float32)
              + np.asarray(inputs["gate_b"], np.float32))
    top2 = np.argpartition(-logits, 1, axis=-1)[:, :2]
    NTOK = x.shape[0]
    QT = NTOK // NQ
    qidx = np.arange(NTOK) // QT
    cnt = np.zeros((cfg["E"], NQ), np.int64)
    np.add.at(cnt, (top2[:, 0], qidx), 1)
    np.add.at(cnt, (top2[:, 1], qidx), 1)
    return int(cnt.max())


_NC_CACHE = {}


def kernel(**inputs) -> np.ndarray:
    import time as _time
    cfg = dict(FULL)
    maps = make_in_maps(inputs, cfg)
    need_b2 = bool(np.any(np.asarray(inputs["b2"], dtype=np.float32)))
    maxq = _routing_max_quarter(inputs, cfg)
    if maxq > cfg["SLOTS_Q"]:  # input drift: rebuild with enough capacity
        cfg["SLOTS_Q"] = ((maxq + 16 + 63) // 64) * 64
    key = (need_b2, cfg["SLOTS_Q"])
    last_err = None
    for attempt in range(4):
        try:
            if _NC_CACHE.get("key") != key:
                _NC_CACHE.clear()
                _NC_CACHE["nc"] = build(cfg, with_b2=need_b2)
                _NC_CACHE["key"] = key
            res = run_bass_kernel_spmd(
                _NC_CACHE["nc"], maps, core_ids=list(range(N_CORES)))
            ys = [np.asarray(res.results[i]["y"]) for i in range(N_CORES)]
            out = unshard_y(ys, cfg)
            # a wedged device can "succeed" with garbage; legitimate outputs
            # for this problem have absmax of a few units
            if not np.isfinite(out).all() or np.abs(out).max() > 1e3:
                raise RuntimeError(
                    f"implausible output (absmax={np.abs(out).max()}), "
                    "retrying on a rebuilt kernel")
            return out
        except Exception as e:  # device wedge / transient runtime failure
            last_err = e
            _NC_CACHE.clear()
            _time.sleep(20 * (attempt + 1))
    raise last_err
